# revision 35
# baseline (speedup 1.0000x reference)
"""CACombiner Trainium2 kernel: conv-projected efficient attention + FFN.

Data-parallel over batch: 8 batch elements -> 8 NeuronCores, identical SPMD
program per core. All heavy matmuls run as float32r (full PE rate); the
attention-weight path (exp(k), v, softmax(q), ctx) runs in bf16.
"""
import sys
sys.path.insert(0, "/opt/trn_rl_repo")
from contextlib import ExitStack

import numpy as np

import concourse.bass as bass
import concourse.tile as tile
from concourse import mybir, bacc
from concourse.bass_utils import run_bass_kernel_spmd
from concourse.alu_op_type import AluOpType

F32 = mybir.dt.float32
F32R = mybir.dt.float32r
BF16 = mybir.dt.bfloat16
AFT = mybir.ActivationFunctionType
Ax = mybir.AxisListType

B, C, L = 8, 512, 4096
H, DK = 8, 64
EPS = 1e-5
CC = C // 128          # 4 channel chunks
NL1 = L // 128         # 32 phase-1 l-tiles
NL2 = L // 512         # 8 phase-2 l-tiles

_CACHE = {}
LAST_RESULT = None


def _build_program():
    nc = bacc.Bacc("TRN2", target_bir_lowering=False, debug=False)

    def din(name, shape, dtype):
        return nc.dram_tensor(name, list(shape), dtype, kind="ExternalInput").ap()

    z1d = din("z1", (C, L), F32R)
    z2d = din("z2", (C, L), F32R)
    WqTt_d = din("WqTt", (128, CC, 512), F32R)
    bq_row_d = din("bq_row", (1, 512), F32R)
    WkvTt_d = din("WkvTt", (128, CC, 1024), F32R)
    WrTt_d = din("WrTt", (128, CC, 512), F32R)
    W1gTt_d = din("W1gTt", (128, CC, 1024), F32R)
    W2gTt_d = din("W2gTt", (128, 8, 512), F32R)
    U1W_d = din("U1W", (2, 1024), F32R)
    u2ct_d = din("u2ct", (128, 8), F32R)
    G2B_d = din("G2B", (2, 512), F32R)
    ivgt_d = din("ivgt", (128, CC), F32R)
    inv512_d = din("inv512", (128, 1), F32R)
    ones1x128_d = din("ones1x128", (1, 128), F32R)
    ident_d = din("ident", (128, 128), BF16)
    br_c_d = din("br_c", (128, CC), F32)
    bv_c_d = din("bv_c", (128, CC), F32)
    be2_c_d = din("be2_c", (128, CC), F32)
    eps_c_d = din("eps_c", (128, 1), F32)
    ones_row_d = din("ones_row", (1, 512), F32R)
    outd = nc.dram_tensor("out", [C, L], F32, kind="ExternalOutput").ap()

    z1r = z1d.rearrange("(cc p) l -> p cc l", p=128)
    z2r = z2d.rearrange("(cc p) l -> p cc l", p=128)

    mm = nc.tensor.matmul
    tt = nc.vector.tensor_tensor
    ts = nc.vector.tensor_scalar
    stt = nc.vector.scalar_tensor_tensor
    act = nc.scalar.activation

    with tile.TileContext(nc) as tc, ExitStack() as ctx:
        cpool = ctx.enter_context(tc.tile_pool(name="consts", bufs=1))

        def const_tile(shape, dtype, src, tag):
            t = cpool.tile(list(shape), dtype, tag=tag, name=tag)
            nc.sync.dma_start(t[:], src)
            return t

        WqTt = const_tile((128, CC, 512), F32R, WqTt_d, "WqTt")
        bq_row = const_tile((1, 512), F32R, bq_row_d, "bq_row")
        WkvTt = const_tile((128, CC, 1024), F32R, WkvTt_d, "WkvTt")
        WrTt = const_tile((128, CC, 512), F32R, WrTt_d, "WrTt")
        W1gTt = const_tile((128, CC, 1024), F32R, W1gTt_d, "W1gTt")
        W2gTt = const_tile((128, 8, 512), F32R, W2gTt_d, "W2gTt")
        U1W = const_tile((2, 1024), F32R, U1W_d, "U1W")
        u2ct = const_tile((128, 8), F32R, u2ct_d, "u2ct")
        G2B = const_tile((2, 512), F32R, G2B_d, "G2B")
        ivgt = const_tile((128, CC), F32R, ivgt_d, "ivgt")
        inv512 = const_tile((128, 1), F32R, inv512_d, "inv512")
        ones1x128 = const_tile((1, 128), F32R, ones1x128_d, "ones1x128")
        ident = const_tile((128, 128), BF16, ident_d, "ident")
        br_c = const_tile((128, CC), F32, br_c_d, "br_c")
        bv_c = const_tile((128, CC), F32, bv_c_d, "bv_c")
        be2_c = const_tile((128, CC), F32, be2_c_d, "be2_c")
        eps_c = const_tile((128, 1), F32, eps_c_d, "eps_c")
        ones_row = const_tile((1, 512), F32R, ones_row_d, "ones_row")

        # persistent across phases
        qsm = cpool.tile([128, CC, L], BF16, tag="qsm", name="qsm")      # softmaxed q, channels-first
        ctxbd = [cpool.tile([128, 128], BF16, tag=f"ctxbd{p}", name=f"ctxbd{p}") for p in range(CC)]

        # ---------------- Phase 1: q softmax + k/v + ctx accumulation ----------------
        with ExitStack() as p1:
            lp1 = p1.enter_context(tc.tile_pool(name="lp1", bufs=2))
            ps_ctx = p1.enter_context(tc.tile_pool(name="ps_ctx", bufs=1, space="PSUM"))
            ps_w = p1.enter_context(tc.tile_pool(name="ps_w", bufs=1, space="PSUM"))

            ctxps = [ps_ctx.tile([128, 129], F32, tag=f"ctx{p}", name=f"ctxps{p}") for p in range(CC)]

            for lt in range(NL1):
                sl = slice(lt * 128, (lt + 1) * 128)
                z1c = lp1.tile([128, CC, 128], F32R, tag="z1c")
                nc.sync.dma_start(z1c[:], z1r[:, :, sl])
                z2c = lp1.tile([128, CC, 128], F32R, tag="z2c")
                nc.sync.dma_start(z2c[:], z2r[:, :, sl])

                # qT [l,128][o,512] = z1^T Wq^T + bq
                qps = ps_w.tile([128, 512], F32, tag="qps")
                for cc in range(CC):
                    mm(qps[:], z1c[:, cc, :], WqTt[:, cc, :], start=(cc == 0), stop=False)
                mm(qps[:], ones1x128[:], bq_row[:], start=False, stop=True)

                # exp + per-head sums (ACT accumulate), then normalize
                EqT = lp1.tile([128, 512], F32, tag="EqT")
                Sq = lp1.tile([128, 8], F32, tag="Sq")
                for h in range(H):
                    hs = slice(h * 64, (h + 1) * 64)
                    act(EqT[:, hs], qps[:, hs], AFT.Exp, accum_out=Sq[:, h:h + 1])
                rq = lp1.tile([128, 8], F32, tag="rq")
                nc.vector.reciprocal(rq[:], Sq[:])
                qsmT = lp1.tile([128, 512], BF16, tag="qsmT")
                tt(qsmT[:].rearrange("p (g x) -> p g x", x=64),
                   EqT[:].rearrange("p (g x) -> p g x", x=64),
                   rq[:].unsqueeze(2).broadcast_to([128, 8, 64]), AluOpType.mult)

                # transpose qsmT back to channels-first into qsm
                tps = ps_w.tile([128, 512], BF16, tag="tps")
                for cc in range(CC):
                    cs = slice(cc * 128, (cc + 1) * 128)
                    nc.tensor.transpose(tps[:, cs], qsmT[:, cs], ident[:])
                nc.vector.tensor_copy(
                    qsm[:, :, sl],
                    tps[:].rearrange("p (cc x) -> p cc x", x=128))

                # kT | vT
                kvps = ps_w.tile([128, 1024], F32, tag="kvps")
                for cc in range(CC):
                    mm(kvps[:, 0:512], z2c[:, cc, :], WkvTt[:, cc, 0:512],
                       start=(cc == 0), stop=(cc == CC - 1))
                for cc in range(CC):
                    mm(kvps[:, 512:1024], z2c[:, cc, :], WkvTt[:, cc, 512:1024],
                       start=(cc == 0), stop=(cc == CC - 1))
                EkT = lp1.tile([128, 512], BF16, tag="EkT")
                act(EkT[:], kvps[:, 0:512], AFT.Exp)
                vT = lp1.tile([128, 516], BF16, tag="vT")
                nc.vector.tensor_copy(
                    vT[:].rearrange("p (pr x) -> p pr x", pr=4)[:, :, 0:128],
                    kvps[:, 512:1024].rearrange("p (pr x) -> p pr x", pr=4))
                nc.vector.memset(vT[:].rearrange("p (pr x) -> p pr x", pr=4)[:, :, 128:129], 1.0)

                # ctx accumulation: per head-pair [2heads-k, 2heads-v | S]
                for pr in range(CC):
                    mm(ctxps[pr][:], EkT[:, pr * 128:(pr + 1) * 128],
                       vT[:, pr * 129:(pr + 1) * 129],
                       start=(lt == 0), stop=(lt == NL1 - 1), skip_group_check=True)

            # finalize ctx: normalize rows by S, build block-diagonal bf16 tiles
            for pr in range(CC):
                rs = lp1.tile([128, 1], F32, tag="rs")
                nc.vector.reciprocal(rs[:], ctxps[pr][:, 128:129])
                nc.vector.memset(ctxbd[pr][:], 0.0)
                ts(ctxbd[pr][0:64, 0:64], ctxps[pr][0:64, 0:64], rs[0:64, :], None,
                   AluOpType.mult)
                ts(ctxbd[pr][64:128, 64:128], ctxps[pr][64:128, 64:128], rs[64:128, :], None,
                   AluOpType.mult)

        # ---------------- Phase 2: attention apply + reprojection + LN/FFN ----------------
        with ExitStack() as p2:
            lp2 = p2.enter_context(tc.tile_pool(name="lp2", bufs=2))
            lph = p2.enter_context(tc.tile_pool(name="lph", bufs=1))
            ps_big = p2.enter_context(tc.tile_pool(name="ps_big", bufs=5, space="PSUM"))
            ps_row = p2.enter_context(tc.tile_pool(name="ps_row", bufs=2, space="PSUM"))

            for lt in range(NL2):
                sl = slice(lt * 512, (lt + 1) * 512)
                z1res = lp2.tile([128, CC, 512], F32R, tag="z1res", bufs=1)
                nc.sync.dma_start(z1res[:], z1r[:, :, sl])

                # att[v,l] = ctx_bd @ qsm + bv
                att = []
                for pr in range(CC):
                    aps = ps_big.tile([128, 512], F32, tag="big")
                    mm(aps[:], ctxbd[pr][:], qsm[:, pr, sl], start=True, stop=True)
                    a = lph.tile([128, 512], F32R, tag=f"att{pr}")
                    ts(a[:], aps[:], bv_c[:, pr:pr + 1], None, AluOpType.add)
                    att.append(a)

                # z = Wr att + br + z1
                zt = []
                for ot in range(CC):
                    zps = ps_big.tile([128, 512], F32, tag="big")
                    for pr in range(CC):
                        mm(zps[:], WrTt[:, pr, ot * 128:(ot + 1) * 128], att[pr][:],
                           start=(pr == 0), stop=(pr == CC - 1))
                    z = lph.tile([128, 512], F32R, tag=f"z{ot}")
                    stt(z[:], zps[:], br_c[:, ot:ot + 1], z1res[:, ot, :].bitcast(F32),
                        AluOpType.add, AluOpType.add)
                    zt.append(z)

                # LN1 stats rows
                mups = ps_row.tile([1, 512], F32, tag="row")
                for ot in range(CC):
                    mm(mups[:], inv512[:], zt[ot][:], start=(ot == 0), stop=(ot == CC - 1))
                e2ps = ps_row.tile([1, 512], F32, tag="row")
                for ot in range(CC):
                    zsq = lp2.tile([128, 512], F32R, tag="zsq")
                    act(zsq[:], zt[ot][:].bitcast(F32), AFT.Square)
                    mm(e2ps[:], inv512[:], zsq[:], start=(ot == 0), stop=(ot == CC - 1))
                murow = lp2.tile([1, 512], F32, tag="murow", bufs=1)
                nc.vector.tensor_copy(murow[:], mups[:])
                musq = lp2.tile([1, 512], F32, tag="musq", bufs=1)
                tt(musq[:], murow[:], murow[:], AluOpType.mult)
                varrow = lp2.tile([1, 512], F32, tag="varrow", bufs=1)
                tt(varrow[:], e2ps[:], musq[:], AluOpType.subtract)
                sig = lp2.tile([1, 512], F32, tag="sig", bufs=1)
                act(sig[:], varrow[:], AFT.Sqrt, bias=eps_c[0:1, :])
                rhs2 = lp2.tile([2, 512], F32R, tag="rhs2", bufs=1)
                ts(rhs2[0:1, :], mups[:], -1.0, None, AluOpType.mult)
                sigR = lp2.tile([1, 512], F32R, tag="sigR", bufs=1)
                nc.vector.tensor_copy(sigR[:], sig[:])
                nc.sync.dma_start(rhs2[1:2, :], sigR[:])
                invsF = lp2.tile([1, 512], F32, tag="invsF", bufs=1)
                nc.vector.reciprocal(invsF[:], sig[:])
                invs = lp2.tile([1, 512], F32R, tag="invs", bufs=1)
                nc.vector.tensor_copy(invs[:], invsF[:])
                bc = ps_big.tile([128, 512], F32, tag="big")
                mm(bc[:], ones1x128[:], invs[:], start=True, stop=True)
                invsb = lp2.tile([128, 512], F32, tag="invsb", bufs=1)
                nc.vector.tensor_copy(invsb[:], bc[:])

                # FFN1 + ELU + FFN2 accumulation (j-outer so hE slots rotate)
                f2ps = [ps_big.tile([128, 512], F32, tag="big", name=f"f2ps{o2}")
                        for o2 in range(CC)]
                mu2 = ps_row.tile([1, 512], F32, tag="row", name="mu2")
                for j in range(8):
                    fps = ps_big.tile([128, 512], F32, tag="big", name="fps")
                    for cc in range(CC):
                        mm(fps[:], W1gTt[:, cc, j * 128:(j + 1) * 128], zt[cc][:],
                           start=(cc == 0), stop=False)
                    mm(fps[:], U1W[:, j * 128:(j + 1) * 128], rhs2[:], start=False, stop=True)
                    hp = lp2.tile([128, 512], F32, tag="hp")
                    tt(hp[:], fps[:], invsb[:], AluOpType.mult)
                    E = lp2.tile([128, 512], F32, tag="E")
                    act(E[:], hp[:], AFT.Exp)
                    nc.gpsimd.tensor_scalar(E[:], E[:], 1.0, -1.0, AluOpType.min,
                                            AluOpType.add)
                    he = lph.tile([128, 512], F32R, tag="hE", bufs=3, name="he")
                    stt(he[:], hp[:], 0.0, E[:], AluOpType.max, AluOpType.add)
                    for o2 in range(CC):
                        mm(f2ps[o2][:], W2gTt[:, j, o2 * 128:(o2 + 1) * 128], he[:],
                           start=(j == 0), stop=False, skip_group_check=True)
                    mm(mu2[:], u2ct[:, j:j + 1], he[:], start=(j == 0), stop=(j == 7),
                       skip_group_check=True)
                rhs2b = lp2.tile([2, 512], F32R, tag="rhs2b", bufs=1)
                nc.sync.dma_start(rhs2b[0:1, :], ones_row[:])
                negmu2 = lp2.tile([1, 512], F32R, tag="negmu2", bufs=1)
                ts(negmu2[:], mu2[:], -1.0, B2MEAN_PLACEHOLDER, AluOpType.mult,
                   AluOpType.subtract)
                nc.sync.dma_start(rhs2b[1:2, :], negmu2[:])
                yg = []
                for o2 in range(CC):
                    mm(f2ps[o2][:], G2B[:, o2 * 128:(o2 + 1) * 128], rhs2b[:],
                       start=False, stop=True, skip_group_check=True)
                    y = lph.tile([128, 512], F32, tag=f"yg{o2}", name=f"yg{o2}")
                    nc.vector.tensor_copy(y[:], f2ps[o2][:])
                    yg.append(y)

                # LN2 variance + apply
                v2ps = ps_row.tile([1, 512], F32, tag="row")
                for o2 in range(CC):
                    sq2 = lp2.tile([128, 512], F32R, tag="sq2")
                    act(sq2[:], yg[o2][:], AFT.Square)
                    mm(v2ps[:], ivgt[:, o2:o2 + 1], sq2[:], start=(o2 == 0),
                       stop=(o2 == CC - 1))
                sig2 = lp2.tile([1, 512], F32, tag="sig2", bufs=1)
                act(sig2[:], v2ps[:], AFT.Sqrt, bias=eps_c[0:1, :])
                invs2F = lp2.tile([1, 512], F32, tag="invs2F", bufs=1)
                nc.vector.reciprocal(invs2F[:], sig2[:])
                invs2 = lp2.tile([1, 512], F32R, tag="invs2", bufs=1)
                nc.vector.tensor_copy(invs2[:], invs2F[:])
                bc2 = ps_big.tile([128, 512], F32, tag="big")
                mm(bc2[:], ones1x128[:], invs2[:], start=True, stop=True)
                invsb2 = lp2.tile([128, 512], F32, tag="invsb2", bufs=1)
                nc.vector.tensor_copy(invsb2[:], bc2[:])
                for o2 in range(CC):
                    tt(yg[o2][:], yg[o2][:], invsb2[:], AluOpType.mult)
                    ot_t = lp2.tile([128, 512], F32, tag="ot")
                    nc.gpsimd.tensor_scalar(ot_t[:], yg[o2][:], be2_c[:, o2:o2 + 1],
                                            None, AluOpType.add)
                    nc.sync.dma_start(outd[o2 * 128:(o2 + 1) * 128, sl], ot_t[:])

    nc.compile()
    return nc


def _prep_consts(Wq, bq, Wk, bk, Wv, bv, Wr, br, g1, be1, W1, b1, W2, b2, g2, be2):
    f = np.float32
    WqT = np.ascontiguousarray(Wq.T, dtype=f)                       # [c, o]
    WkvT = np.concatenate([Wk.T, Wv.T], axis=1).astype(f)           # [c, k|v]
    WrT = np.ascontiguousarray(Wr.T, dtype=f)                       # [v, o]
    W1g = (W1 * g1[None, :]).astype(f)                              # [1024, c]
    W1gT = np.ascontiguousarray(W1g.T)                              # [c, 1024]
    W2g = (W2 * g2[:, None]).astype(f)                              # [c, 1024h]
    W2gT = np.ascontiguousarray(W2g.T)                              # [h, c]
    u1 = W1g.sum(axis=1).astype(f)
    w1bb = (W1 @ be1 + b1).astype(f)
    u2 = (W2.sum(axis=0) / 512.0).astype(f)
    ivg = (1.0 / (512.0 * g2 * g2)).astype(f)
    b2mean = float(np.mean(b2))

    def chunkT(a, n):          # [n*128, m] -> [128, n, m]
        return np.ascontiguousarray(a.reshape(n, 128, -1).transpose(1, 0, 2))

    def colsT(v, n):           # [n*128] -> [128, n]
        return np.ascontiguousarray(v.reshape(n, 128).T)

    return {
        "WqTt": chunkT(WqT, CC),
        "bq_row": bq.reshape(1, 512).astype(f),
        "WkvTt": chunkT(WkvT, CC),
        "WrTt": chunkT(WrT, CC),
        "W1gTt": chunkT(W1gT, CC),
        "W2gTt": chunkT(W2gT, 8),
        "U1W": np.stack([u1, w1bb]).astype(f),
        "u2ct": colsT(u2, 8),
        "G2B": np.stack([(g2 * b2).astype(f), g2.astype(f)]),
        "ivgt": colsT(ivg, CC),
        "inv512": np.full((128, 1), 1.0 / 512.0, dtype=f),
        "ones1x128": np.ones((1, 128), dtype=f),
        "ident": np.eye(128, dtype=f).astype(np.dtype("bfloat16") if False else f),
        "br_c": colsT(br.astype(f), CC),
        "bv_c": colsT(bv.astype(f), CC),
        "be2_c": colsT(be2.astype(f), CC),
        "eps_c": np.full((128, 1), EPS, dtype=f),
        "ones_row": np.ones((1, 512), dtype=f),
    }, b2mean


def kernel(**inputs):
    global LAST_RESULT
    import ml_dtypes
    z1 = np.asarray(inputs["z1"], dtype=np.float32)
    z2 = np.asarray(inputs["z2"], dtype=np.float32)
    consts, b2mean = _prep_consts(
        *[np.asarray(inputs[k], dtype=np.float32) for k in
          ["Wq", "bq", "Wk", "bk", "Wv", "bv", "Wr", "br", "g1", "be1",
           "W1", "b1", "W2", "b2", "g2", "be2"]])
    consts["ident"] = np.eye(128, dtype=ml_dtypes.bfloat16)

    key = ("prog", round(b2mean * 1e9))
    if key not in _CACHE:
        global B2MEAN_PLACEHOLDER
        B2MEAN_PLACEHOLDER = b2mean
        _CACHE.clear()
        _CACHE[key] = _build_program()
    nc = _CACHE[key]

    in_maps = []
    for b in range(B):
        m = dict(consts)
        m["z1"] = np.ascontiguousarray(z1[b])
        m["z2"] = np.ascontiguousarray(z2[b])
        in_maps.append(m)

    import os
    trace = bool(int(os.environ.get("KERNEL_TRACE", "0")))
    res = run_bass_kernel_spmd(nc, in_maps, list(range(B)), trace=trace)
    LAST_RESULT = res
    out = np.stack([res.results[b]["out"] for b in range(B)], axis=0)
    return out.astype(np.float32)


B2MEAN_PLACEHOLDER = 0.0

